# revision 9
# baseline (speedup 1.0000x reference)
"""AdaptiveMixing Trainium2 kernel — 8-core data parallel.

Math (per query n):
  q  = LayerNorm(query[n]) * ln_w + ln_b
  h  = q @ w1.T + b1                      # [128]
  params = h @ w2.T + b2                  # [66560]
  cm = params[:65536].reshape(256, 256)   # [c, d]
  sm = params[65536:].reshape(32, 32)     # [op, p]
  o1 = gelu(x[n] @ cm + m_beta)           # [32, 256]
  o2 = gelu(sm @ o1 + s_beta[:, None])    # [32, 256]
  out[n] = o2.reshape(8192) @ proj_w.T + proj_b

Design: w2 streamed once per core in d-quarter epochs; each [64,128] w2
slab is PE-transposed so the contraction dim (k=128) lands on partitions;
cm for all 256 local queries is generated into PSUM as [c, m] tiles with a
strided stationary operand picking {t = c*256 + d0}, drained to SBUF in
bf16 with the b2 bias fused, then consumed by 4-way column-tiled per-query
mixing matmuls. proj_w is streamed/transposed per f-tile during the two
d-half epochs and accumulated in held PSUM banks.
"""

import sys

sys.path.insert(0, "/opt/trn_rl_repo")

import numpy as np

import concourse.bass as bass
import concourse.mybir as mybir
import concourse.tile as tile
from concourse.bass_utils import run_bass_kernel_spmd
from concourse.masks import make_identity

F32 = mybir.dt.float32
BF16 = mybir.dt.bfloat16
AF = mybir.ActivationFunctionType

B, N, P, C = 2, 1024, 32, 256
OP, HID = 32, 128
TOTAL = C * C + OP * P  # 66560
NCORES = 8
M = (B * N) // NCORES  # 256 queries per core
NG = M // 4            # 64 query groups of 4


def _dram_ap(handle, offset, ap):
    return bass.AP(tensor=handle.ap().tensor, offset=offset, ap=[list(p) for p in ap])


def build(nc: bass.Bass):
    # ---- I/O ----
    d_query = nc.dram_tensor("query", [M, C], F32, kind="ExternalInput")
    d_x = nc.dram_tensor("x", [M, P, C], F32, kind="ExternalInput")
    d_lnw = nc.dram_tensor("ln_w", [C], F32, kind="ExternalInput")
    d_lnb = nc.dram_tensor("ln_b", [C], F32, kind="ExternalInput")
    d_w1 = nc.dram_tensor("w1", [HID, C], F32, kind="ExternalInput")
    d_b1 = nc.dram_tensor("b1", [HID], F32, kind="ExternalInput")
    d_w2 = nc.dram_tensor("w2", [TOTAL, HID], F32, kind="ExternalInput")
    d_b2 = nc.dram_tensor("b2", [TOTAL], F32, kind="ExternalInput")
    d_mb = nc.dram_tensor("m_beta", [C], F32, kind="ExternalInput")
    d_sb = nc.dram_tensor("s_beta", [OP], F32, kind="ExternalInput")
    d_pw = nc.dram_tensor("proj_w", [C, OP * C], F32, kind="ExternalInput")
    d_pb = nc.dram_tensor("proj_b", [C], F32, kind="ExternalInput")
    d_y = nc.dram_tensor("y", [M, C], F32, kind="ExternalOutput")

    from contextlib import ExitStack
    with tile.TileContext(nc) as tc, ExitStack() as ctx:
        _build_body(ctx, nc, tc, d_query, d_x, d_lnw, d_lnb, d_w1, d_b1, d_w2,
                    d_b2, d_mb, d_sb, d_pw, d_pb, d_y)
    return nc


def _build_body(ctx, nc, tc, d_query, d_x, d_lnw, d_lnb, d_w1, d_b1, d_w2,
                d_b2, d_mb, d_sb, d_pw, d_pb, d_y):
    singles = ctx.enter_context(tc.tile_pool(name="singles", bufs=1))
    ps_t = ctx.enter_context(tc.tile_pool(name="ps_t", bufs=2, space="PSUM"))
    ps_cm = ctx.enter_context(tc.tile_pool(name="ps_cm", bufs=2, space="PSUM"))
    ps_mm = ctx.enter_context(tc.tile_pool(name="ps_mm", bufs=2, space="PSUM"))
    ps_out = ctx.enter_context(tc.tile_pool(name="ps_out", bufs=1, space="PSUM"))
    tmp3 = ctx.enter_context(tc.tile_pool(name="tmp3", bufs=3))
    stage2 = ctx.enter_context(tc.tile_pool(name="stage2", bufs=2))

    # ---- constants ----
    ident = singles.tile([128, 128], F32)
    make_identity(nc, ident)
    ident_bf = singles.tile([128, 128], BF16)
    nc.vector.tensor_copy(out=ident_bf, in_=ident)

    lnw_b = singles.tile([128, C], F32)
    nc.sync.dma_start(out=lnw_b, in_=_dram_ap(d_lnw, 0, [[0, 128], [1, C]]))
    lnb_b = singles.tile([128, C], F32)
    nc.sync.dma_start(out=lnb_b, in_=_dram_ap(d_lnb, 0, [[0, 128], [1, C]]))
    mbeta_b = singles.tile([128, C], F32)
    nc.sync.dma_start(out=mbeta_b, in_=_dram_ap(d_mb, 0, [[0, 128], [1, C]]))
    b1_sb = singles.tile([128, 1], F32)
    nc.sync.dma_start(out=b1_sb, in_=_dram_ap(d_b1, 0, [[1, 128], [0, 1]]))
    projb_sb = singles.tile([128, 2], F32)
    nc.sync.dma_start(out=projb_sb, in_=_dram_ap(d_pb, 0, [[1, 128], [128, 2]]))
    eps_sb = singles.tile([128, 1], F32)
    nc.vector.memset(eps_sb, 1e-6)
    # s_beta broadcast over 4-query row blocks: part (r, op) -> s_beta[op]
    sbeta_bd = singles.tile([128, 1], F32)
    for r in range(4):
        nc.sync.dma_start(out=sbeta_bd[32 * r:32 * r + 32, :],
                          in_=_dram_ap(d_sb, 0, [[1, 32], [0, 1]]))
    # b2 for cm: [c_low, ch, d0] ; b2 for sm: [p, op]
    b2cm = singles.tile([128, 2, C], F32)
    nc.sync.dma_start(out=b2cm, in_=_dram_ap(d_b2, 0, [[C, 128], [128 * C, 2], [1, C]]))
    b2sm = singles.tile([32, 32], F32)
    nc.sync.dma_start(out=b2sm, in_=_dram_ap(d_b2, C * C, [[1, 32], [32, 32]]))

    # ---- LayerNorm on queries ([m, c], m on partitions) ----
    qn = singles.tile([128, 2, C], F32)
    for mt in range(2):
        qt = qn[:, mt, :]
        nc.sync.dma_start(out=qt, in_=_dram_ap(d_query, mt * 128 * C, [[C, 128], [1, C]]))
        stats = tmp3.tile([128, 6], F32)
        nc.vector.bn_stats(out=stats, in_=qt)
        mv = tmp3.tile([128, 2], F32)
        nc.vector.bn_aggr(out=mv, in_=stats)
        rstd = tmp3.tile([128, 1], F32)
        nc.scalar.activation(out=rstd, in_=mv[:, 1:2], func=AF.Sqrt,
                             bias=eps_sb, scale=1.0)
        nc.vector.reciprocal(out=rstd, in_=rstd)
        nc.vector.tensor_scalar(out=qt, in0=qt, scalar1=mv[:, 0:1], scalar2=rstd,
                                op0=mybir.AluOpType.subtract, op1=mybir.AluOpType.mult)
        nc.vector.tensor_mul(out=qt, in0=qt, in1=lnw_b)
        nc.vector.tensor_add(out=qt, in0=qt, in1=lnb_b)

    # qnT [c_low, ch, m] bf16
    qnT = singles.tile([128, 2, M], BF16)
    for mt in range(2):
        for ch in range(2):
            pt = ps_t.tile([128, 128], F32, tag="pt")
            nc.tensor.transpose(pt, qn[:, mt, 128 * ch:128 * (ch + 1)], ident)
            nc.vector.tensor_copy(out=qnT[:, ch, 128 * mt:128 * (mt + 1)], in_=pt)

    # w1T [c_low, ch, k] bf16
    w1_sb = tmp3.tile([128, C], F32, tag="w1")
    nc.sync.dma_start(out=w1_sb, in_=_dram_ap(d_w1, 0, [[C, 128], [1, C]]))
    w1T = singles.tile([128, 2, 128], BF16)
    for ch in range(2):
        pt = ps_t.tile([128, 128], F32, tag="pt")
        nc.tensor.transpose(pt, w1_sb[:, 128 * ch:128 * (ch + 1)], ident)
        nc.vector.tensor_copy(out=w1T[:, ch, :], in_=pt)

    # hT [k, m] = w1 @ qn.T + b1   (bf16 copy for later matmuls)
    ps_h = ps_cm.tile([128, M], F32, tag="cmps")
    for ch in range(2):
        nc.tensor.matmul(ps_h, w1T[:, ch, :], qnT[:, ch, :],
                         start=(ch == 0), stop=(ch == 1))
    hT = singles.tile([128, M], BF16)
    nc.vector.tensor_scalar_add(out=hT, in0=ps_h, scalar1=b1_sb)

    # ---- sm params: w2 rows [65536, 66560) ----
    w2Tsm = singles.tile([128, 1024], BF16)
    for i in range(8):
        st = stage2.tile([128, 128], F32, tag="w2sm")
        nc.sync.dma_start(out=st, in_=_dram_ap(
            d_w2, (C * C + i * 128) * HID, [[HID, 128], [1, HID]]))
        pt = ps_t.tile([128, 128], F32, tag="pt")
        nc.tensor.transpose(pt, st, ident)
        nc.vector.tensor_copy(out=w2Tsm[:, 128 * i:128 * (i + 1)], in_=pt)

    # smT_all [128=(r,p), g, 128=(r,op)] block-diagonal per 4-query group
    smT_all = singles.tile([128, NG, 128], BF16)
    nc.gpsimd.memset(smT_all, 0.0)
    for op in range(OP):
        ps_sm = ps_mm.tile([32, M], F32, tag="mmps")
        nc.tensor.matmul(ps_sm, w2Tsm[:, 32 * op:32 * (op + 1)], hT,
                         start=True, stop=True)
        src = ps_sm.rearrange("p (g r) -> p g r", r=4)
        for r in range(4):
            nc.vector.tensor_scalar_add(
                out=smT_all[32 * r:32 * r + 32, :, 32 * r + op],
                in0=src[:, :, r], scalar1=b2sm[:, op:op + 1])

    # ---- xsT [c_low, ch, g, (4q x 32p)] bf16 ----
    xsT = singles.tile([128, 2, NG, 128], BF16)
    for g in range(NG):
        xt = tmp3.tile([128, C], F32, tag="x4")
        nc.sync.dma_start(out=xt, in_=_dram_ap(d_x, g * 128 * C, [[C, 128], [1, C]]))
        for ch in range(2):
            pt = ps_t.tile([128, 128], F32, tag="pt")
            nc.tensor.transpose(pt, xt[:, 128 * ch:128 * (ch + 1)], ident)
            nc.vector.tensor_copy(out=xsT[:, ch, g, :], in_=pt)

    # ---- big persistent buffers ----
    w2Tq = singles.tile([128, C, 64], BF16)      # [k, c, j] for one d-quarter
    cm_sb = singles.tile([128, 2, 64, M], BF16)  # [c_low, ch, j, m]
    flat_sb = singles.tile([128, 32, M], BF16)   # [d_low, op, m] for one d-half
    ps_acc0 = ps_out.tile([128, M], F32, tag="acc0")
    ps_acc1 = ps_out.tile([128, M], F32, tag="acc1")
    ps_acc = [ps_acc0, ps_acc1]

    # ---- main loop over d-quarters ----
    for dq in range(4):
        dh, parity = dq // 2, dq % 2
        # stage w2: rows t = c*256 + dq*64 + j
        for cb in range(16):
            st = stage2.tile([64, 16, HID], F32, tag="w2st")
            nc.sync.dma_start(out=st, in_=_dram_ap(
                d_w2, (cb * 16 * C + dq * 64) * HID,
                [[HID, 64], [C * HID, 16], [1, HID]]))
            for ci in range(16):
                c = cb * 16 + ci
                pt = ps_t.tile([128, 64], F32, tag="pt")
                nc.tensor.transpose(pt, st[:, ci, :], ident[:64, :64])
                if c % 2 == 0:
                    nc.vector.tensor_copy(out=w2Tq[:, c, :], in_=pt)
                else:
                    nc.scalar.activation(out=w2Tq[:, c, :], in_=pt,
                                         func=AF.Copy, scale=1.0)
        # cm matmuls: psum [c_low, m] at (d0=dq*64+j, ch)
        for j in range(64):
            for ch in range(2):
                pc = ps_cm.tile([128, M], F32, tag="cmps")
                lhsT = w2Tq[:, 128 * ch:128 * (ch + 1), j]
                nc.tensor.matmul(pc, lhsT, hT, start=True, stop=True)
                d0 = dq * 64 + j
                bias = b2cm[:, ch, d0:d0 + 1]
                if j % 2 == 0:
                    nc.vector.tensor_scalar_add(out=cm_sb[:, ch, j, :], in0=pc,
                                                scalar1=bias)
                else:
                    nc.scalar.activation(out=cm_sb[:, ch, j, :], in_=pc,
                                         func=AF.Identity, bias=bias, scale=1.0)
        # mixing per 4-query group
        for g in range(NG):
            pm1 = ps_mm.tile([128, 64], F32, tag="mmps")
            for ch in range(2):
                for jq in range(4):
                    rhs = cm_sb[:, ch, :, 4 * g + jq]
                    nc.tensor.matmul(
                        pm1[32 * jq:32 * jq + 32, :],
                        xsT[:, ch, g, 32 * jq:32 * jq + 32], rhs,
                        start=(ch == 0), stop=(ch == 1),
                        tile_position=(0, 32 * jq))
            m1t = tmp3.tile([128, 64], F32, tag="m1t")
            nc.vector.tensor_add(out=m1t, in0=pm1,
                                 in1=mbeta_b[:, dq * 64:(dq + 1) * 64])
            m1g = tmp3.tile([128, 64], BF16, tag="m1g")
            nc.scalar.activation(out=m1g, in_=m1t, func=AF.Gelu, scale=1.0)
            pm2 = ps_mm.tile([128, 64], F32, tag="mmps")
            nc.tensor.matmul(pm2, smT_all[:, g, :], m1g, start=True, stop=True)
            o2g = tmp3.tile([128, 64], BF16, tag="o2g")
            nc.scalar.activation(out=o2g, in_=pm2, func=AF.Gelu,
                                 bias=sbeta_bd, scale=1.0)
            # transpose to flat rows [d_low, (r, op)]
            pt2 = ps_t.tile([64, 128], BF16, tag="pt")
            nc.tensor.transpose(pt2, o2g, ident_bf)
            dst = flat_sb[64 * parity:64 * parity + 64, :, 4 * g:4 * g + 4]
            dst = dst.rearrange("d o r -> d r o")
            src = pt2.rearrange("d (r o) -> d r o", o=32)
            nc.vector.tensor_copy(out=dst, in_=src)
        # proj accumulation at end of each d-half
        if parity == 1:
            for op in range(OP):
                kt = op * 2 + dh
                pw = stage2.tile([128, 2, 128], F32, tag="pwst")
                pwT = stage2.tile([128, C], BF16, tag="pwT")
                for et in range(2):
                    nc.sync.dma_start(out=pw[:, et, :], in_=_dram_ap(
                        d_pw, et * 128 * OP * C + kt * 128,
                        [[OP * C, 128], [1, 128]]))
                    pt = ps_t.tile([128, 128], F32, tag="pt")
                    nc.tensor.transpose(pt, pw[:, et, :], ident)
                    nc.scalar.activation(out=pwT[:, 128 * et:128 * (et + 1)],
                                         in_=pt, func=AF.Copy, scale=1.0)
                for et in range(2):
                    nc.tensor.matmul(ps_acc[et], pwT[:, 128 * et:128 * (et + 1)],
                                     flat_sb[:, op, :],
                                     start=(dh == 0 and op == 0),
                                     stop=(dh == 1 and op == OP - 1))

    # ---- epilogue: bias, transpose to [m, e], store ----
    outE = singles.tile([128, 2, M], F32)
    for et in range(2):
        nc.scalar.activation(out=outE[:, et, :], in_=ps_acc[et], func=AF.Identity,
                             bias=projb_sb[:, et:et + 1], scale=1.0)
    out_sb = singles.tile([128, 2, C], F32)
    for mt in range(2):
        for et in range(2):
            pt = ps_t.tile([128, 128], F32, tag="pt")
            nc.tensor.transpose(pt, outE[:, et, 128 * mt:128 * (mt + 1)], ident)
            nc.vector.tensor_copy(out=out_sb[:, mt, 128 * et:128 * (et + 1)], in_=pt)
    for mt in range(2):
        nc.sync.dma_start(out=_dram_ap(d_y, mt * 128 * C, [[C, 128], [1, C]]),
                          in_=out_sb[:, mt, :])


def legalize_sync_waits(nc, max_waits=1):
    """This walrus build accepts only one sync wait per instruction; move
    extras onto preceding same-engine NoOps."""
    ctr = 0
    for f in nc.m.functions:
        for bb in f.blocks:
            out, changed = [], False
            for inst in bb.instructions:
                si = inst.sync_info
                if si is not None and si.on_wait and len(si.on_wait) > max_waits:
                    waits = list(si.on_wait)
                    for w in waits[:-max_waits]:
                        ctr += 1
                        n = mybir.InstNoOp(name=f"lw_nop_{ctr}", ins=[], outs=[])
                        n.engine = inst.engine
                        n.sync_info = mybir.SyncInfo(on_update=[], on_wait=[w])
                        out.append(n)
                    inst.sync_info = mybir.SyncInfo(
                        on_update=list(si.on_update or []),
                        on_wait=waits[-max_waits:])
                    changed = True
                out.append(inst)
            if changed:
                bb.instructions = out
    return ctr


_CACHE = {}


def _get_nc():
    if "nc" not in _CACHE:
        nc = bass.Bass()
        build(nc)
        legalize_sync_waits(nc)
        _CACHE["nc"] = nc
    return _CACHE["nc"]


def kernel(**inputs):
    nc = _get_nc()
    x = np.ascontiguousarray(np.asarray(inputs["x"], dtype=np.float32)
                             .reshape(B * N, P, C))
    query = np.ascontiguousarray(np.asarray(inputs["query"], dtype=np.float32)
                                 .reshape(B * N, C))
    shared = {}
    for k in ("ln_w", "ln_b", "w1", "b1", "w2", "b2", "m_beta", "s_beta",
              "proj_w", "proj_b"):
        shared[k] = np.ascontiguousarray(np.asarray(inputs[k], dtype=np.float32))
    in_maps = []
    for c in range(NCORES):
        m = dict(shared)
        m["x"] = np.ascontiguousarray(x[c * M:(c + 1) * M])
        m["query"] = np.ascontiguousarray(query[c * M:(c + 1) * M])
        in_maps.append(m)
    res = run_bass_kernel_spmd(nc, in_maps, core_ids=list(range(NCORES)))
    out = np.concatenate([res.results[c]["y"] for c in range(NCORES)], axis=0)
    return out.reshape(B, N, C)


if __name__ == "__main__":
    rng = np.random.default_rng(0)
    ins = {
        "x": rng.standard_normal((B, N, 1, P, C), dtype=np.float32),
        "query": rng.standard_normal((B, N, C), dtype=np.float32),
        "ln_w": np.full((C,), C ** -0.5, np.float32),
        "ln_b": np.zeros((C,), np.float32),
        "w1": (rng.standard_normal((HID, C)) * 0.02).astype(np.float32),
        "b1": np.zeros((HID,), np.float32),
        "w2": (rng.standard_normal((TOTAL, HID)) * 0.02).astype(np.float32),
        "b2": (rng.standard_normal((TOTAL,)) * 0.05).astype(np.float32),
        "m_beta": np.zeros((C,), np.float32),
        "s_beta": np.zeros((OP,), np.float32),
        "proj_w": (rng.standard_normal((C, OP * C)) * 0.02).astype(np.float32),
        "proj_b": np.zeros((C,), np.float32),
    }
    out = kernel(**ins)
    print("ran", out.shape, out.dtype)


# revision 15
# speedup vs baseline: 58.9181x; 58.9181x over previous
"""AdaptiveMixing Trainium2 kernel — 8-core data parallel.

Math (per query n):
  q  = LayerNorm(query[n]) * ln_w + ln_b
  h  = q @ w1.T + b1                      # [128]
  params = h @ w2.T + b2                  # [66560]
  cm = params[:65536].reshape(256, 256)   # [c, d]
  sm = params[65536:].reshape(32, 32)     # [op, p]
  o1 = gelu(x[n] @ cm + m_beta)           # [32, 256]
  o2 = gelu(sm @ o1 + s_beta[:, None])    # [32, 256]
  out[n] = o2.reshape(8192) @ proj_w.T + proj_b

Design: w2 streamed once per core in d-quarter epochs; each [64,128] w2
slab is PE-transposed so the contraction dim (k=128) lands on partitions;
cm for all 256 local queries is generated into PSUM as [c, m] tiles with a
strided stationary operand picking {t = c*256 + d0}, drained to SBUF in
bf16 with the b2 bias fused, then consumed by 4-way column-tiled per-query
mixing matmuls. proj_w is streamed/transposed per f-tile during the two
d-half epochs and accumulated in held PSUM banks.
"""

import sys

sys.path.insert(0, "/opt/trn_rl_repo")

import numpy as np

import concourse.bass as bass
import concourse.mybir as mybir
import concourse.tile as tile
from concourse.bass_utils import run_bass_kernel_spmd
from concourse.masks import make_identity

F32 = mybir.dt.float32
BF16 = mybir.dt.bfloat16
AF = mybir.ActivationFunctionType

B, N, P, C = 2, 1024, 32, 256
OP, HID = 32, 128
TOTAL = C * C + OP * P  # 66560
NCORES = 8
M = (B * N) // NCORES  # 256 queries per core
NG = M // 4            # 64 query groups of 4


def _dram_ap(handle, offset, ap):
    return bass.AP(tensor=handle.ap().tensor, offset=offset, ap=[list(p) for p in ap])


def build(nc: bass.Bass):
    # ---- I/O ----
    d_query = nc.dram_tensor("query", [M, C], F32, kind="ExternalInput")
    d_x = nc.dram_tensor("x", [M, P, C], F32, kind="ExternalInput")
    d_lnw = nc.dram_tensor("ln_w", [C], F32, kind="ExternalInput")
    d_lnb = nc.dram_tensor("ln_b", [C], F32, kind="ExternalInput")
    d_w1 = nc.dram_tensor("w1", [HID, C], F32, kind="ExternalInput")
    d_b1 = nc.dram_tensor("b1", [HID], F32, kind="ExternalInput")
    d_w2 = nc.dram_tensor("w2", [TOTAL, HID], F32, kind="ExternalInput")
    d_b2 = nc.dram_tensor("b2", [TOTAL], F32, kind="ExternalInput")
    d_mb = nc.dram_tensor("m_beta", [C], F32, kind="ExternalInput")
    d_sb = nc.dram_tensor("s_beta", [OP], F32, kind="ExternalInput")
    d_pw = nc.dram_tensor("proj_w", [C, OP * C], F32, kind="ExternalInput")
    d_pb = nc.dram_tensor("proj_b", [C], F32, kind="ExternalInput")
    d_y = nc.dram_tensor("y", [M, C], F32, kind="ExternalOutput")

    from contextlib import ExitStack
    with tile.TileContext(nc) as tc, ExitStack() as ctx:
        _build_body(ctx, nc, tc, d_query, d_x, d_lnw, d_lnb, d_w1, d_b1, d_w2,
                    d_b2, d_mb, d_sb, d_pw, d_pb, d_y)
    return nc


def _build_body(ctx, nc, tc, d_query, d_x, d_lnw, d_lnb, d_w1, d_b1, d_w2,
                d_b2, d_mb, d_sb, d_pw, d_pb, d_y):
    import os
    SKIP_MIX = bool(int(os.environ.get("AK_SKIP_MIX", "0")))
    SKIP_CM = bool(int(os.environ.get("AK_SKIP_CM", "0")))
    singles = ctx.enter_context(tc.tile_pool(name="singles", bufs=1))
    ps_t = ctx.enter_context(tc.tile_pool(name="ps_t", bufs=2, space="PSUM"))
    ps_cm = ctx.enter_context(tc.tile_pool(name="ps_cm", bufs=2, space="PSUM"))
    ps_mm = ctx.enter_context(tc.tile_pool(name="ps_mm", bufs=2, space="PSUM"))
    ps_out = ctx.enter_context(tc.tile_pool(name="ps_out", bufs=1, space="PSUM"))
    tmp3 = ctx.enter_context(tc.tile_pool(name="tmp3", bufs=3))
    stage2 = ctx.enter_context(tc.tile_pool(name="stage2", bufs=2))

    # ---- constants ----
    ident = singles.tile([128, 128], F32)
    make_identity(nc, ident)
    ident_bf = singles.tile([128, 128], BF16)
    nc.vector.tensor_copy(out=ident_bf, in_=ident)

    lnw_b = singles.tile([128, C], F32)
    nc.sync.dma_start(out=lnw_b, in_=_dram_ap(d_lnw, 0, [[0, 128], [1, C]]))
    lnb_b = singles.tile([128, C], F32)
    nc.sync.dma_start(out=lnb_b, in_=_dram_ap(d_lnb, 0, [[0, 128], [1, C]]))
    mbeta_b = singles.tile([128, C], F32)
    nc.sync.dma_start(out=mbeta_b, in_=_dram_ap(d_mb, 0, [[0, 128], [1, C]]))
    b1_sb = singles.tile([128, 1], F32)
    nc.sync.dma_start(out=b1_sb, in_=_dram_ap(d_b1, 0, [[1, 128], [0, 1]]))
    projb_sb = singles.tile([128, 2], F32)
    nc.sync.dma_start(out=projb_sb, in_=_dram_ap(d_pb, 0, [[1, 128], [128, 2]]))
    eps_sb = singles.tile([128, 1], F32)
    nc.vector.memset(eps_sb, 1e-6)
    # s_beta broadcast over 4-query row blocks: part (r, op) -> s_beta[op]
    sbeta_bd = singles.tile([128, 1], F32)
    for r in range(4):
        nc.sync.dma_start(out=sbeta_bd[32 * r:32 * r + 32, :],
                          in_=_dram_ap(d_sb, 0, [[1, 32], [0, 1]]))
    # b2 for cm: [c_low, ch, d0] ; b2 for sm: [p, op]
    b2cm = singles.tile([128, 2, C], F32)
    nc.sync.dma_start(out=b2cm, in_=_dram_ap(d_b2, 0, [[C, 128], [128 * C, 2], [1, C]]))
    b2sm = singles.tile([32, 32], F32)
    nc.sync.dma_start(out=b2sm, in_=_dram_ap(d_b2, C * C, [[1, 32], [32, 32]]))

    # ---- LayerNorm on queries ([m, c], m on partitions) ----
    qn = singles.tile([128, 2, C], F32)
    for mt in range(2):
        qt = qn[:, mt, :]
        nc.sync.dma_start(out=qt, in_=_dram_ap(d_query, mt * 128 * C, [[C, 128], [1, C]]))
        stats = tmp3.tile([128, 6], F32)
        nc.vector.bn_stats(out=stats, in_=qt)
        mv = tmp3.tile([128, 2], F32)
        nc.vector.bn_aggr(out=mv, in_=stats)
        rstd = tmp3.tile([128, 1], F32)
        nc.scalar.activation(out=rstd, in_=mv[:, 1:2], func=AF.Sqrt,
                             bias=eps_sb, scale=1.0)
        nc.vector.reciprocal(out=rstd, in_=rstd)
        nc.vector.tensor_scalar(out=qt, in0=qt, scalar1=mv[:, 0:1], scalar2=rstd,
                                op0=mybir.AluOpType.subtract, op1=mybir.AluOpType.mult)
        nc.vector.tensor_mul(out=qt, in0=qt, in1=lnw_b)
        nc.vector.tensor_add(out=qt, in0=qt, in1=lnb_b)

    # qnT [c_low, ch, m] bf16
    qnT = singles.tile([128, 2, M], BF16)
    for mt in range(2):
        for ch in range(2):
            pt = ps_t.tile([128, 128], F32, tag="pt")
            nc.tensor.transpose(pt, qn[:, mt, 128 * ch:128 * (ch + 1)], ident)
            nc.vector.tensor_copy(out=qnT[:, ch, 128 * mt:128 * (mt + 1)], in_=pt)

    # w1T [c_low, ch, k] bf16
    w1_sb = tmp3.tile([128, C], F32, tag="w1")
    nc.sync.dma_start(out=w1_sb, in_=_dram_ap(d_w1, 0, [[C, 128], [1, C]]))
    w1T = singles.tile([128, 2, 128], BF16)
    for ch in range(2):
        pt = ps_t.tile([128, 128], F32, tag="pt")
        nc.tensor.transpose(pt, w1_sb[:, 128 * ch:128 * (ch + 1)], ident)
        nc.vector.tensor_copy(out=w1T[:, ch, :], in_=pt)

    # hT [k, m] = w1 @ qn.T + b1   (bf16 copy for later matmuls)
    ps_h = ps_cm.tile([128, M], F32, tag="cmps")
    for ch in range(2):
        nc.tensor.matmul(ps_h, w1T[:, ch, :], qnT[:, ch, :],
                         start=(ch == 0), stop=(ch == 1))
    hT = singles.tile([128, M], BF16)
    nc.vector.tensor_scalar_add(out=hT, in0=ps_h, scalar1=b1_sb)

    # ---- sm params: w2 rows [65536, 66560) ----
    w2Tsm = singles.tile([128, 1024], BF16)
    for i in range(8):
        st = stage2.tile([128, 128], F32, tag="w2sm")
        nc.sync.dma_start(out=st, in_=_dram_ap(
            d_w2, (C * C + i * 128) * HID, [[HID, 128], [1, HID]]))
        pt = ps_t.tile([128, 128], F32, tag="pt")
        nc.tensor.transpose(pt, st, ident)
        nc.vector.tensor_copy(out=w2Tsm[:, 128 * i:128 * (i + 1)], in_=pt)

    # smT_all [128=(r,p), g, 128=(r,op)] block-diagonal per 4-query group
    smT_all = singles.tile([128, NG, 128], BF16)
    nc.gpsimd.memset(smT_all, 0.0)
    for op in range(OP):
        ps_sm = ps_mm.tile([32, M], F32, tag="mmps")
        nc.tensor.matmul(ps_sm, w2Tsm[:, 32 * op:32 * (op + 1)], hT,
                         start=True, stop=True)
        src = ps_sm.rearrange("p (g r) -> p g r", r=4)
        for r in range(4):
            nc.vector.tensor_scalar_add(
                out=smT_all[32 * r:32 * r + 32, :, 32 * r + op],
                in0=src[:, :, r], scalar1=b2sm[:, op:op + 1])

    # ---- xsT [c_low, ch, g, (4q x 32p)] bf16 ----
    xsT = singles.tile([128, 2, NG, 128], BF16)
    for g in range(NG):
        xt = tmp3.tile([128, C], F32, tag="x4")
        nc.sync.dma_start(out=xt, in_=_dram_ap(d_x, g * 128 * C, [[C, 128], [1, C]]))
        for ch in range(2):
            pt = ps_t.tile([128, 128], F32, tag="pt")
            nc.tensor.transpose(pt, xt[:, 128 * ch:128 * (ch + 1)], ident)
            nc.vector.tensor_copy(out=xsT[:, ch, g, :], in_=pt)

    # ---- big persistent buffers ----
    w2Tq = singles.tile([128, C, 64], BF16)      # [k, c, j] for one d-quarter
    cm_sb = singles.tile([128, 2, 64, M], BF16)  # [c_low, ch, j, m]
    flat_sb = singles.tile([128, 32, M], BF16)   # [d_low, op, m] for one d-half
    ps_acc0 = ps_out.tile([128, M], F32, tag="acc0")
    ps_acc1 = ps_out.tile([128, M], F32, tag="acc1")
    ps_acc = [ps_acc0, ps_acc1]

    # ---- main loop over d-quarters ----
    for dq in range(4):
        dh, parity = dq // 2, dq % 2
        # stage w2: rows t = c*256 + dq*64 + j
        for cb in range(16):
            st = stage2.tile([64, 16, HID], F32, tag="w2st")
            nc.sync.dma_start(out=st, in_=_dram_ap(
                d_w2, (cb * 16 * C + dq * 64) * HID,
                [[HID, 64], [C * HID, 16], [1, HID]]))
            for ci in range(16):
                c = cb * 16 + ci
                pt = ps_t.tile([128, 64], F32, tag="pt")
                nc.tensor.transpose(pt, st[:, ci, :], ident[:64, :64])
                if c % 2 == 0:
                    nc.vector.tensor_copy(out=w2Tq[:, c, :], in_=pt)
                else:
                    nc.scalar.activation(out=w2Tq[:, c, :], in_=pt,
                                         func=AF.Copy, scale=1.0)
        # cm matmuls: psum [c_low, m] at (d0=dq*64+j, ch)
        for j in range(64 if not SKIP_CM else 0):
            for ch in range(2):
                pc = ps_cm.tile([128, M], F32, tag="cmps")
                lhsT = w2Tq[:, 128 * ch:128 * (ch + 1), j]
                nc.tensor.matmul(pc, lhsT, hT, start=True, stop=True)
                d0 = dq * 64 + j
                bias = b2cm[:, ch, d0:d0 + 1]
                if j % 2 == 0:
                    nc.vector.tensor_scalar_add(out=cm_sb[:, ch, j, :], in0=pc,
                                                scalar1=bias)
                else:
                    nc.scalar.activation(out=cm_sb[:, ch, j, :], in_=pc,
                                         func=AF.Identity, bias=bias, scale=1.0)
        # mixing per 4-query group
        for g in range(NG if not SKIP_MIX else 0):
            pm1 = ps_mm.tile([128, 64], F32, tag="mmps")
            for ch in range(2):
                for jq in range(4):
                    rhs = cm_sb[:, ch, :, 4 * g + jq]
                    nc.tensor.matmul(
                        pm1[32 * jq:32 * jq + 32, :],
                        xsT[:, ch, g, 32 * jq:32 * jq + 32], rhs,
                        start=(ch == 0), stop=(ch == 1),
                        tile_position=(0, 32 * jq))
            m1t = tmp3.tile([128, 64], F32, tag="m1t")
            nc.vector.tensor_add(out=m1t, in0=pm1,
                                 in1=mbeta_b[:, dq * 64:(dq + 1) * 64])
            m1g = tmp3.tile([128, 64], BF16, tag="m1g")
            nc.scalar.activation(out=m1g, in_=m1t, func=AF.Gelu, scale=1.0)
            pm2 = ps_mm.tile([128, 64], F32, tag="mmps")
            nc.tensor.matmul(pm2, smT_all[:, g, :], m1g, start=True, stop=True)
            o2g = tmp3.tile([128, 64], BF16, tag="o2g")
            nc.scalar.activation(out=o2g, in_=pm2, func=AF.Gelu,
                                 bias=sbeta_bd, scale=1.0)
            # transpose to flat rows [d_low, (r, op)]
            pt2 = ps_t.tile([64, 128], BF16, tag="pt")
            nc.tensor.transpose(pt2, o2g, ident_bf)
            dst = flat_sb[64 * parity:64 * parity + 64, :, 4 * g:4 * g + 4]
            dst = dst.rearrange("d o r -> d r o")
            src = pt2.rearrange("d (r o) -> d r o", o=32)
            nc.vector.tensor_copy(out=dst, in_=src)
        # proj accumulation at end of each d-half
        if parity == 1 and not SKIP_MIX:
            for op in range(OP):
                kt = op * 2 + dh
                pw = stage2.tile([128, 2, 128], F32, tag="pwst")
                pwT = stage2.tile([128, C], BF16, tag="pwT")
                for et in range(2):
                    nc.sync.dma_start(out=pw[:, et, :], in_=_dram_ap(
                        d_pw, et * 128 * OP * C + kt * 128,
                        [[OP * C, 128], [1, 128]]))
                    pt = ps_t.tile([128, 128], F32, tag="pt")
                    nc.tensor.transpose(pt, pw[:, et, :], ident)
                    nc.scalar.activation(out=pwT[:, 128 * et:128 * (et + 1)],
                                         in_=pt, func=AF.Copy, scale=1.0)
                for et in range(2):
                    nc.tensor.matmul(ps_acc[et], pwT[:, 128 * et:128 * (et + 1)],
                                     flat_sb[:, op, :],
                                     start=(dh == 0 and op == 0),
                                     stop=(dh == 1 and op == OP - 1))

    # ---- epilogue: bias, transpose to [m, e], store ----
    outE = singles.tile([128, 2, M], F32)
    for et in range(2):
        nc.scalar.activation(out=outE[:, et, :], in_=ps_acc[et], func=AF.Identity,
                             bias=projb_sb[:, et:et + 1], scale=1.0)
    out_sb = singles.tile([128, 2, C], F32)
    for mt in range(2):
        for et in range(2):
            pt = ps_t.tile([128, 128], F32, tag="pt")
            nc.tensor.transpose(pt, outE[:, et, 128 * mt:128 * (mt + 1)], ident)
            nc.vector.tensor_copy(out=out_sb[:, mt, 128 * et:128 * (et + 1)], in_=pt)
    for mt in range(2):
        nc.sync.dma_start(out=_dram_ap(d_y, mt * 128 * C, [[C, 128], [1, C]]),
                          in_=out_sb[:, mt, :])


def legalize_sync_waits(nc, max_waits=1):
    """This walrus build accepts only one sync wait per instruction; move
    extras onto preceding same-engine NoOps."""
    ctr = 0
    for f in nc.m.functions:
        for bb in f.blocks:
            out, changed = [], False
            for inst in bb.instructions:
                si = inst.sync_info
                if si is not None and si.on_wait and len(si.on_wait) > max_waits:
                    waits = list(si.on_wait)
                    for w in waits[:-max_waits]:
                        ctr += 1
                        n = mybir.InstNoOp(name=f"lw_nop_{ctr}", ins=[], outs=[])
                        n.engine = inst.engine
                        n.sync_info = mybir.SyncInfo(on_update=[], on_wait=[w])
                        out.append(n)
                    inst.sync_info = mybir.SyncInfo(
                        on_update=list(si.on_update or []),
                        on_wait=waits[-max_waits:])
                    changed = True
                out.append(inst)
            if changed:
                bb.instructions = out
    return ctr


_CACHE = {}


def _get_nc():
    if "nc" not in _CACHE:
        nc = bass.Bass()
        build(nc)
        legalize_sync_waits(nc)
        _CACHE["nc"] = nc
    return _CACHE["nc"]


def kernel(**inputs):
    nc = _get_nc()
    x = np.ascontiguousarray(np.asarray(inputs["x"], dtype=np.float32)
                             .reshape(B * N, P, C))
    query = np.ascontiguousarray(np.asarray(inputs["query"], dtype=np.float32)
                                 .reshape(B * N, C))
    shared = {}
    for k in ("ln_w", "ln_b", "w1", "b1", "w2", "b2", "m_beta", "s_beta",
              "proj_w", "proj_b"):
        shared[k] = np.ascontiguousarray(np.asarray(inputs[k], dtype=np.float32))
    in_maps = []
    for c in range(NCORES):
        m = dict(shared)
        m["x"] = np.ascontiguousarray(x[c * M:(c + 1) * M])
        m["query"] = np.ascontiguousarray(query[c * M:(c + 1) * M])
        in_maps.append(m)
    res = run_bass_kernel_spmd(nc, in_maps, core_ids=list(range(NCORES)))
    out = np.concatenate([res.results[c]["y"] for c in range(NCORES)], axis=0)
    return out.reshape(B, N, C)


if __name__ == "__main__":
    rng = np.random.default_rng(0)
    ins = {
        "x": rng.standard_normal((B, N, 1, P, C), dtype=np.float32),
        "query": rng.standard_normal((B, N, C), dtype=np.float32),
        "ln_w": np.full((C,), C ** -0.5, np.float32),
        "ln_b": np.zeros((C,), np.float32),
        "w1": (rng.standard_normal((HID, C)) * 0.02).astype(np.float32),
        "b1": np.zeros((HID,), np.float32),
        "w2": (rng.standard_normal((TOTAL, HID)) * 0.02).astype(np.float32),
        "b2": (rng.standard_normal((TOTAL,)) * 0.05).astype(np.float32),
        "m_beta": np.zeros((C,), np.float32),
        "s_beta": np.zeros((OP,), np.float32),
        "proj_w": (rng.standard_normal((C, OP * C)) * 0.02).astype(np.float32),
        "proj_b": np.zeros((C,), np.float32),
    }
    out = kernel(**ins)
    print("ran", out.shape, out.dtype)
